# revision 24
# baseline (speedup 1.0000x reference)
"""Trainium2 Bass kernel for nn_MultiHeadAttention_31542239822105.

Math (faithful to reference, incl. softmax over the QUERY axis):
  q = einsum('bsd,hde->bhse', x, Wq) + bq ; same k, v
  scores = q @ k^T * 1/sqrt(DH)          [B,H,Sq,Sk]
  probs  = softmax(scores, axis=2)       # over q (query axis!)
  ctx    = einsum('bhqk,bhke->bhqe', probs, v)
  out    = ctx.reshape(B,S,D) @ Wo + bo
Sharding: data-parallel over batch, 8 cores x 8 batch items. No collectives.

Per-core layout (all matmul contraction dims land on partitions):
  - x is pre-transposed on the HOST to xT [D, tokens].
  - Q^T,K^T projections come out f-major; V token-major.
  - scoresT[k,q] per head -> softmax over q is a FREE-axis reduction.
  - 1/denominator folded into V rows; ctxT accumulates f-major per head
    pair; output projection emits token-major, direct DMA out.
  - 1/sqrt(DH) folded into Wq/bq on the host.

Schedule: one globally software-pipelined instruction stream built from
"slots", one slot per (head-pair, key-chunk) scores tile:
  - PE: 4 scores matmuls into a PSUM pair tile [128, 2*577] (head pair
    side by side), + ctx matmuls of the slot 2 back, + a cycle-quota of
    projection matmuls for the NEXT batch item (and the output projection
    of the PREVIOUS item). The PE stream never waits on softmax: it ramps
    to the 2.4GHz p-state and stays there.
  - Scalar: ONE Exp activation per slot over the 1154-wide pair tile
    (amortizes the 352-cycle ACT startup), + Q-proj evictions (bias via
    per-partition ACT bias) + ctxT evictions (Copy).
  - DVE: per-slot denominator tensor_reduce [128,2,577]->[128,2] on bf16
    probs, reciprocal, K/V/out-proj evictions.
  - GpSimd (otherwise idle): folds 1/den into the V rows (SBUF->SBUF).
PSUM: scores pair pool 3 banks + proj pair pool 3 banks + ctx 2 banks = 8.
"""

import sys

if "/opt/trn_rl_repo" not in sys.path:
    sys.path.insert(0, "/opt/trn_rl_repo")

from collections import deque

import numpy as np
import ml_dtypes

import concourse.bass as bass
import concourse.mybir as mybir
import concourse.tile as tile_mod
from concourse.vector_clock import ScopedClock
from concourse.bass_utils import run_bass_kernel_spmd

# ---------------------------------------------------------------- constants
B, S, D, H = 64, 577, 768, 12
DH = D // H          # 64
NCORES = 8
BC = B // NCORES     # 8 batch items per core
DC = D // 128        # 6 d-chunks
FC = D // 128        # 6 f-chunks per projection matrix
M_QK = 2 * FC        # 12 combined Q+K f-chunks
TT = (S + 127) // 128  # 5 token tiles (128,128,128,128,65)
NP = H // 2          # 6 head pairs
S2 = 2 * S           # 1154: scores pair tile width

BF16 = mybir.dt.bfloat16
F32 = mybir.dt.float32
nbf = ml_dtypes.bfloat16

_TILE_PATCHED = False
_CUR_NC = [None]


def _patch_tile_drain():
    """The walrus build here rejects >1 sync-wait per instruction
    ("Too many sync wait commands"). Two patches:
    1. post-legalize pass that moves extra waits onto single-wait nops
       inserted just before the offending instruction (same engine);
    2. the final SP Drain (emitted after legalize) gets the same split.
    """
    global _TILE_PATCHED
    if _TILE_PATCHED:
        return
    _TILE_PATCHED = True

    _orig_postorder = tile_mod.postorder_instruction_blocks

    def _split_multi_waits(ordered, nc):
        for bbname, insts in ordered.items():
            out = []
            n_split = 0
            for inst in insts:
                si = inst.sync_info
                if si is not None and len(si.on_wait) > 1:
                    waits = list(si.on_wait)
                    for w in waits[:-1]:
                        nop = mybir.InstNoOp(
                            name=nc.get_next_instruction_name(),
                            ins=[],
                            outs=[],
                            bass_is_fusable=False,
                        )
                        nop.engine = inst.engine
                        nop.sync_info = mybir.SyncInfo(on_wait=[w], on_update=[])
                        nc.register_instruction(nop, overwrite=True)
                        out.append(nop)
                        n_split += 1
                    inst.sync_info = mybir.SyncInfo(
                        on_wait=[waits[-1]], on_update=list(si.on_update)
                    )
                out.append(inst)
            ordered[bbname] = out
        return ordered

    def postorder_and_split(ordered, start_bb, postordered):
        nc = _CUR_NC[0]
        _split_multi_waits(ordered, nc)
        return _orig_postorder(ordered, start_bb, postordered)

    tile_mod.postorder_instruction_blocks = postorder_and_split

    def _drain_and_barrier_split(self, tick_clock, wait_clock):
        nc = self.nc
        drain_inst = nc.sync.drain()
        wait_clock.add_sem_waits(
            drain_inst.ins, ScopedClock({None: tick_clock.global_clock})
        )
        si = drain_inst.ins.sync_info
        waits = list(si.on_wait)
        if len(waits) > 1:
            drain_inst.ins.sync_info = mybir.SyncInfo(
                on_wait=[waits[0]], on_update=list(si.on_update)
            )
            for w in waits[1:]:
                nop = nc.sync.nop(nofuse=True)
                nop.ins.sync_info = mybir.SyncInfo(on_wait=[w], on_update=[])
        nc.all_engine_barrier()
        assert self.sems is not None
        popped = nc._tile_sem_poison_stack.pop()
        assert popped is self._sem_poison
        nc.clear_and_free_semaphores(list(self.sems.allocated().values()))
        nc.all_engine_barrier()

    tile_mod.TileContext._drain_and_barrier = _drain_and_barrier_split


# ---------------------------------------------------------------- builder
def build_bass(bc=BC):
    """Emit the per-core kernel for `bc` batch items. Returns nc."""
    _patch_tile_drain()
    nc = bass.Bass()
    _CUR_NC[0] = nc

    xt_d = nc.declare_dram_parameter("xt", [DC, 128, bc, S], BF16, isOutput=False)
    wqk_d = nc.declare_dram_parameter("wqk", [128, M_QK, DC, 128], BF16, isOutput=False)
    wv_d = nc.declare_dram_parameter("wv", [128, DC, D], BF16, isOutput=False)
    wo_d = nc.declare_dram_parameter("wo", [128, FC, D], BF16, isOutput=False)
    bqk_d = nc.declare_dram_parameter("bqk", [128, M_QK], F32, isOutput=False)
    bvbc_d = nc.declare_dram_parameter("bvbc", [128, D], F32, isOutput=False)
    bobc_d = nc.declare_dram_parameter("bobc", [128, D], F32, isOutput=False)
    out_d = nc.declare_dram_parameter("out", [bc, S, D], F32, isOutput=True)

    AF = mybir.ActivationFunctionType
    AX = mybir.AxisListType
    OP = mybir.AluOpType

    with tile_mod.TileContext(nc) as tc:
        with (
            tc.tile_pool(name="singles", bufs=1) as singles,
            tc.tile_pool(name="xt", bufs=3) as xpool,
            tc.tile_pool(name="qk", bufs=2) as qkpool,
            tc.tile_pool(name="v", bufs=2) as vpool,
            tc.tile_pool(name="probs", bufs=10) as ppool,
            tc.tile_pool(name="den", bufs=2) as dpool,
            tc.tile_pool(name="dab", bufs=2) as dabpool,
            tc.tile_pool(name="rd", bufs=2) as rpool,
            tc.tile_pool(name="vszp", bufs=2) as vzpool,
            tc.tile_pool(name="ctxT", bufs=2) as cpool,
            tc.tile_pool(name="ot", bufs=4) as opool,
            tc.tile_pool(name="psA", bufs=1, space="PSUM") as psA,
            tc.tile_pool(name="psB", bufs=1, space="PSUM") as psB,
            tc.tile_pool(name="psC", bufs=1, space="PSUM") as psC,
        ):
            # -------- resident weights / biases
            wqk = singles.tile([128, M_QK, DC, 128], BF16)
            nc.sync.dma_start(out=wqk, in_=wqk_d[:])
            wv = singles.tile([128, DC, D], BF16)
            nc.sync.dma_start(out=wv, in_=wv_d[:])
            wo = singles.tile([128, FC, D], BF16)
            nc.sync.dma_start(out=wo, in_=wo_d[:])
            bqk = singles.tile([128, M_QK], F32)
            nc.sync.dma_start(out=bqk, in_=bqk_d[:])
            bvbc = singles.tile([128, D], F32)
            nc.sync.dma_start(out=bvbc, in_=bvbc_d[:])
            bobc = singles.tile([128, D], F32)
            nc.sync.dma_start(out=bobc, in_=bobc_d[:])

            # ktz: K^T zero-PADDED per head so the scores lhsT is a full
            # 128-partition operand. Two dedicated tiles (item b uses
            # ktz[b%2]); the pad halves are zeroed ONCE here and never
            # rewritten (K evictions only touch their own half).
            ktz = [singles.tile([128, FC, 2, S], BF16, name=f"ktz{i}")
                   for i in range(2)]
            for kt in ktz:
                nc.vector.memset(kt[64:128, :, 0, :], 0.0)
                nc.vector.memset(kt[0:64, :, 1, :], 0.0)

            # prime the exp table-set load before the pipeline starts
            warm = singles.tile([128, 2], F32)
            nc.vector.memset(warm[:, 0:1], 0.0)
            nc.scalar.activation(warm[:, 1:2], warm[:, 0:1], AF.Exp)

            # -------- per-item SBUF tile handles
            xts, qks, vs, ctxTs = {}, {}, {}, {}

            def load_xt(b):
                xt = xpool.tile([128, DC, S], BF16, tag="xt")
                for dc in range(DC):
                    nc.sync.dma_start(out=xt[:, dc, :], in_=xt_d[dc, :, b, :])
                xts[b] = xt

            def alloc_item(b):
                qks[b] = qkpool.tile([128, FC, S], BF16, tag="qk", name="qk")
                vs[b] = vpool.tile([128, TT, D], BF16, tag="v", name="v")
                ctxTs[b] = cpool.tile([128, FC, S], BF16, tag="ctx", name="ctx")

            # -------- projection generators (yield per-matmul cycle cost)
            def gen_qkproj(b):
                # NOTE: a matmul with start=True invalidates its ENTIRE
                # 2KB PSUM bank, so every accumulation region must own its
                # banks exclusively -> one [128,768] (2-bank) tile per chunk.
                xt, qk, ktzb = xts[b], qks[b], ktz[b % 2]
                for m in range(FC):
                    ps = psB.tile([128, 768], F32, tag="psB")
                    for dc in range(DC):
                        st, sp = dc == 0, dc == DC - 1
                        for (r0, r1) in ((0, 512), (512, 577)):
                            nc.tensor.matmul(
                                ps[:, r0:r1], lhsT=wqk[:, m, dc, :],
                                rhs=xt[:, dc, r0:r1], start=st, stop=sp)
                            yield r1 - r0
                    # evict Q (per-partition bias + bf16 cast on ScalarE)
                    nc.scalar.activation(
                        qk[:, m, :], ps[:, 0:S], AF.Identity,
                        bias=bqk[:, m : m + 1], scale=1.0)
                    yield 0
                    ps = psB.tile([128, 768], F32, tag="psB")
                    for dc in range(DC):
                        st, sp = dc == 0, dc == DC - 1
                        for (r0, r1) in ((0, 512), (512, 577)):
                            nc.tensor.matmul(
                                ps[:, r0:r1], lhsT=wqk[:, FC + m, dc, :],
                                rhs=xt[:, dc, r0:r1], start=st, stop=sp)
                            yield r1 - r0
                    # evict K halves into the zero-padded layout (ScalarE,
                    # per-partition bias rides the activation)
                    nc.scalar.activation(
                        ktzb[0:64, m, 0, :], ps[0:64, 0:S], AF.Identity,
                        bias=bqk[0:64, FC + m : FC + m + 1], scale=1.0)
                    nc.scalar.activation(
                        ktzb[64:128, m, 1, :], ps[64:128, 0:S], AF.Identity,
                        bias=bqk[64:128, FC + m : FC + m + 1], scale=1.0)
                    yield 0

            def gen_tokproj(b, kind):
                # kind 'v': V = xT.T @ Wv ; kind 'o': out = ctxT.T @ Wo
                if kind == "v":
                    lhs_src, rhs_w, nred = xts[b], wv, DC
                else:
                    lhs_src, rhs_w, nred = ctxTs[b], wo, FC
                for tt in range(TT):
                    t0 = tt * 128
                    tsz = min(128, S - t0)
                    ps = psB.tile([128, 768], F32, tag="psB")
                    for rc in range(nred):
                        st, sp = rc == 0, rc == nred - 1
                        for (r0, r1) in ((0, 512), (512, 768)):
                            nc.tensor.matmul(
                                ps[:tsz, r0:r1],
                                lhsT=lhs_src[:, rc, t0 : t0 + tsz],
                                rhs=rhs_w[:, rc, r0:r1], start=st, stop=sp)
                            yield r1 - r0
                    if kind == "v":
                        nc.vector.tensor_add(
                            vs[b][:tsz, tt, :], ps[:tsz, 0:D], bvbc[:tsz])
                    else:
                        ot = opool.tile([128, D], F32, tag="ot")
                        nc.vector.tensor_add(
                            ot[:tsz], ps[:tsz, 0:D], bobc[:tsz])
                        nc.sync.dma_start(
                            out=out_d[b, t0 : t0 + tsz, :], in_=ot[:tsz])
                    yield 0

            proj_gens = deque()

            def drain_proj(quota):
                cy = 0
                while proj_gens and cy < quota:
                    try:
                        cy += next(proj_gens[0])
                    except StopIteration:
                        proj_gens.popleft()
                return cy

            # -------- attention slot machinery
            # Slot (pair p, key-chunk kc): 4 scores MMs -> paired Exp
            # (accum gives denA+denB on ScalarE) -> one denA reduce (DVE).
            # Pair-end: ONE sub (denB for all 5 chunks), ONE reciprocal,
            # and TWO GpSimd tensor_muls that scale the pair's V columns
            # by 1/den (rd broadcast via stride-0 AP). The ctx matmuls run
            # TT+3 slots behind; each head writes its own 64-partition
            # half of the ctx PSUM tile, so no zero-padding of V at all.
            mm_queue = deque()
            psc_box = [None]
            pair_box = [None]
            slot_idx = [0]

            def make_ctx_task(b, p, kc, probs, vszP, ksz):
                def emit():
                    if kc == 0:
                        psc_box[0] = psC.tile([128, S], F32, tag="psC", name="psc")
                    psc = psc_box[0]
                    st, sp = kc == 0, kc == TT - 1
                    for hh in (0, 1):
                        po = hh * 64
                        for (r0, r1) in ((0, 512), (512, 577)):
                            nc.tensor.matmul(
                                psc[po : po + 64, r0:r1],
                                lhsT=vszP[:ksz, kc, po : po + 64],
                                rhs=probs[:ksz, hh, r0:r1], start=st, stop=sp)
                    if kc == TT - 1:
                        nc.vector.tensor_copy(ctxTs[b][:, p, :], psc[:, 0:S])
                return emit

            def emit_slot(b, p, kc, quota):
                g = slot_idx[0]
                slot_idx[0] += 1
                ksz = min(128, S - kc * 128)
                k0 = kc * 128
                qkb, ktzb, vb = qks[b], ktz[b % 2], vs[b]
                # scores pair tile: head A at bank 0, head B at bank 2 (a
                # start=True matmul invalidates its whole 2KB bank, so the
                # two heads' regions must be bank-disjoint)
                ps = psA.tile([128, 2, 1024], F32, tag="psA")
                for (hh, r0, r1) in (
                    (0, 0, 512), (0, 512, 577), (1, 0, 512), (1, 512, 577),
                ):
                    nc.tensor.matmul(
                        ps[:ksz, hh, r0:r1],
                        lhsT=ktzb[:, p, hh, k0 : k0 + ksz],
                        rhs=qkb[:, p, r0:r1], start=True, stop=True)
                if kc == 0:
                    den = dpool.tile([128, TT, 2], F32, tag="den")
                    dab = dabpool.tile([128, TT], F32, tag="dab")
                    rdp = rpool.tile([128, TT, 2], F32, tag="rd")
                    pair_box[0] = (den, dab, rdp)
                den, dab, rdp = pair_box[0]
                # one Exp over both heads; accumulator gives denA+denB
                probs = ppool.tile([128, 2, S], BF16, tag="probs")
                nc.scalar.activation(
                    probs[:ksz, :, :], ps[:ksz, :, 0:S], AF.Exp,
                    accum_out=dab[:ksz, kc : kc + 1])
                # lagged ctx matmuls (from completed pairs)
                while mm_queue and mm_queue[0][0] <= g - (TT + 3):
                    mm_queue.popleft()[1]()
                # proj evictions must precede the Exp-dependent reduce in
                # the in-order DVE stream (they gate single-buffered psB)
                drain_proj(quota)
                nc.vector.tensor_reduce(
                    den[:ksz, kc, 0:1], probs[:ksz, 0, :], axis=AX.X, op=OP.add)
                if kc == TT - 1:
                    nc.vector.tensor_sub(
                        den[:, :, 1], dab[:, :], den[:, :, 0])
                    nc.vector.reciprocal(rdp[:, :, :], den[:, :, :])
                    vszP = vzpool.tile([128, TT, 128], BF16, tag="vszP")
                    c0 = 2 * p * DH
                    nc.gpsimd.tensor_mul(
                        vszP[:, :, 0:64], vb[:, :, c0 : c0 + DH],
                        rdp[:, :, 0:1].broadcast_to((128, TT, DH)))
                    nc.gpsimd.tensor_mul(
                        vszP[:, :, 64:128], vb[:, :, c0 + DH : c0 + 2 * DH],
                        rdp[:, :, 1:2].broadcast_to((128, TT, DH)))
                    pair_box[1:] = [(probs, vszP)]
                pair_probs[kc] = (probs, ksz)
                if kc == TT - 1:
                    vszP = pair_box[1][1]
                    for k2 in range(TT):
                        pr, ks2 = pair_probs[k2]
                        mm_queue.append(
                            (g - (TT - 1) + k2,
                             make_ctx_task(b, p, k2, pr, vszP, ks2)))

            pair_probs = {}

            # -------- prologue: item 0 projections run un-overlapped
            load_xt(0)
            if bc > 1:
                load_xt(1)
            alloc_item(0)
            proj_gens.append(gen_qkproj(0))
            proj_gens.append(gen_tokproj(0, "v"))
            drain_proj(1 << 30)

            # -------- main pipeline
            for b in range(bc):
                # flush item b-1's remaining ctx tasks BEFORE pushing the
                # out-projection that reads ctxT[b-1]: a reader emitted
                # before its writer cannot be ordered by the dep tracker
                while mm_queue:
                    mm_queue.popleft()[1]()
                if b + 2 < bc:
                    load_xt(b + 2)
                if b + 1 < bc:
                    alloc_item(b + 1)
                total = 0
                if b > 0:
                    proj_gens.append(gen_tokproj(b - 1, "o"))
                    total += FC * TT * D  # out-proj: 23040 cy
                if b + 1 < bc:
                    proj_gens.append(gen_qkproj(b + 1))
                    proj_gens.append(gen_tokproj(b + 1, "v"))
                    total += M_QK * DC * S + DC * TT * D  # 41544 + 23040
                quota = total // (NP * TT) + 40
                for p in range(NP):
                    for kc in range(TT):
                        emit_slot(b, p, kc, quota)
                drain_proj(1 << 30)

            # -------- epilogue
            while mm_queue:
                mm_queue.popleft()[1]()
            proj_gens.append(gen_tokproj(bc - 1, "o"))
            drain_proj(1 << 30)

    return nc


# ---------------------------------------------------------------- host prep
def _prep_shared(Wq, bq, Wk, bk, Wv, bv, Wo, bo):
    """Build the per-core-identical weight operands."""
    scale = np.float32(1.0 / np.sqrt(DH))
    wqf = (Wq.astype(np.float32) * scale).transpose(1, 0, 2).reshape(D, D)
    wkf = Wk.astype(np.float32).transpose(1, 0, 2).reshape(D, D)
    wvf = Wv.astype(np.float32).transpose(1, 0, 2).reshape(D, D)

    def chunk4(wf):  # [d, f] -> [di, m, dc, fi]
        return wf.reshape(DC, 128, FC, 128).transpose(1, 2, 0, 3)

    wqk = np.concatenate([chunk4(wqf), chunk4(wkf)], axis=1)  # [128, 12, 6, 128]
    wv3 = wvf.reshape(DC, 128, D).transpose(1, 0, 2)          # [128, 6, 768]
    wo3 = Wo.astype(np.float32).reshape(FC, 128, D).transpose(1, 0, 2)

    bqf = (bq.astype(np.float32) * scale).reshape(D)
    bkf = bk.astype(np.float32).reshape(D)
    bqk = np.concatenate(
        [bqf.reshape(FC, 128), bkf.reshape(FC, 128)], axis=0
    ).T.copy()                                                # [128, 12]
    bvbc = np.broadcast_to(bv.astype(np.float32).reshape(D), (128, D)).copy()
    bobc = np.broadcast_to(bo.astype(np.float32).reshape(D), (128, D)).copy()

    return {
        "wqk": np.ascontiguousarray(wqk).astype(nbf),
        "wv": np.ascontiguousarray(wv3).astype(nbf),
        "wo": np.ascontiguousarray(wo3).astype(nbf),
        "bqk": np.ascontiguousarray(bqk),
        "bvbc": bvbc,
        "bobc": bobc,
    }


_NC_CACHE = {}


def kernel(x, Wq, bq, Wk, bk, Wv, bv, Wo, bo):
    x = np.asarray(x, dtype=np.float32)
    shared = _prep_shared(
        np.asarray(Wq), np.asarray(bq), np.asarray(Wk), np.asarray(bk),
        np.asarray(Wv), np.asarray(bv), np.asarray(Wo), np.asarray(bo))

    in_maps = []
    for c in range(NCORES):
        xc = x[c * BC : (c + 1) * BC]                    # [BC, S, D]
        xt = xc.transpose(2, 0, 1)                       # [D, BC, S]
        xt = xt.reshape(DC, 128, BC, S).astype(nbf)
        m = dict(shared)
        m["xt"] = np.ascontiguousarray(xt)
        in_maps.append(m)

    if "nc" not in _NC_CACHE:
        _NC_CACHE["nc"] = build_bass()
    nc = _NC_CACHE["nc"]

    res = run_bass_kernel_spmd(nc, in_maps, core_ids=list(range(NCORES)))
    out = np.concatenate([res.results[c]["out"] for c in range(NCORES)], axis=0)
    return out.astype(np.float32)


if __name__ == "__main__":
    rng = np.random.default_rng(0)
    ins = {
        "x": rng.standard_normal((B, S, D), dtype=np.float32),
        "Wq": rng.standard_normal((H, D, DH), dtype=np.float32) * 0.02,
        "bq": np.zeros((H, DH), np.float32),
        "Wk": rng.standard_normal((H, D, DH), dtype=np.float32) * 0.02,
        "bk": np.zeros((H, DH), np.float32),
        "Wv": rng.standard_normal((H, D, DH), dtype=np.float32) * 0.02,
        "bv": np.zeros((H, DH), np.float32),
        "Wo": rng.standard_normal((D, D), dtype=np.float32) * 0.02,
        "bo": np.zeros((D,), np.float32),
    }
    o = kernel(**ins)
    print("out", o.shape, o.dtype, float(np.abs(o).max()))


# revision 25
# speedup vs baseline: 1.1457x; 1.1457x over previous
"""Trainium2 Bass kernel for nn_MultiHeadAttention_31542239822105.

Math (faithful to reference, incl. softmax over the QUERY axis):
  q = einsum('bsd,hde->bhse', x, Wq) + bq ; same k, v
  scores = q @ k^T * 1/sqrt(DH)          [B,H,Sq,Sk]
  probs  = softmax(scores, axis=2)       # over q (query axis!)
  ctx    = einsum('bhqk,bhke->bhqe', probs, v)
  out    = ctx.reshape(B,S,D) @ Wo + bo
Sharding: data-parallel over batch, 8 cores x 8 batch items. No collectives.

Per-core layout (all matmul contraction dims land on partitions):
  - x is pre-transposed on the HOST to xT [D, tokens].
  - Q^T,K^T projections come out f-major; V token-major.
  - scoresT[k,q] per head -> softmax over q is a FREE-axis reduction.
  - 1/denominator folded into V rows; ctxT accumulates f-major per head
    pair; output projection emits token-major, direct DMA out.
  - 1/sqrt(DH) folded into Wq/bq on the host.

Schedule: one globally software-pipelined instruction stream built from
"slots", one slot per (head-pair, key-chunk) scores tile:
  - PE: 4 scores matmuls into a PSUM pair tile [128, 2*577] (head pair
    side by side), + ctx matmuls of the slot 2 back, + a cycle-quota of
    projection matmuls for the NEXT batch item (and the output projection
    of the PREVIOUS item). The PE stream never waits on softmax: it ramps
    to the 2.4GHz p-state and stays there.
  - Scalar: ONE Exp activation per slot over the 1154-wide pair tile
    (amortizes the 352-cycle ACT startup), + Q-proj evictions (bias via
    per-partition ACT bias) + ctxT evictions (Copy).
  - DVE: per-slot denominator tensor_reduce [128,2,577]->[128,2] on bf16
    probs, reciprocal, K/V/out-proj evictions.
  - GpSimd (otherwise idle): folds 1/den into the V rows (SBUF->SBUF).
PSUM: scores pair pool 3 banks + proj pair pool 3 banks + ctx 2 banks = 8.
"""

import sys

if "/opt/trn_rl_repo" not in sys.path:
    sys.path.insert(0, "/opt/trn_rl_repo")

from collections import deque

import numpy as np
import ml_dtypes

import concourse.bass as bass
import concourse.mybir as mybir
import concourse.tile as tile_mod
from concourse.vector_clock import ScopedClock
from concourse.bass_utils import run_bass_kernel_spmd

# ---------------------------------------------------------------- constants
B, S, D, H = 64, 577, 768, 12
DH = D // H          # 64
NCORES = 8
BC = B // NCORES     # 8 batch items per core
DC = D // 128        # 6 d-chunks
FC = D // 128        # 6 f-chunks per projection matrix
M_QK = 2 * FC        # 12 combined Q+K f-chunks
TT = (S + 127) // 128  # 5 token tiles (128,128,128,128,65)
NP = H // 2          # 6 head pairs
S2 = 2 * S           # 1154: scores pair tile width

BF16 = mybir.dt.bfloat16
F32 = mybir.dt.float32
nbf = ml_dtypes.bfloat16

_TILE_PATCHED = False
_CUR_NC = [None]


def _patch_tile_drain():
    """The walrus build here rejects >1 sync-wait per instruction
    ("Too many sync wait commands"). Two patches:
    1. post-legalize pass that moves extra waits onto single-wait nops
       inserted just before the offending instruction (same engine);
    2. the final SP Drain (emitted after legalize) gets the same split.
    """
    global _TILE_PATCHED
    if _TILE_PATCHED:
        return
    _TILE_PATCHED = True

    _orig_postorder = tile_mod.postorder_instruction_blocks

    def _split_multi_waits(ordered, nc):
        for bbname, insts in ordered.items():
            out = []
            n_split = 0
            for inst in insts:
                si = inst.sync_info
                if si is not None and len(si.on_wait) > 1:
                    waits = list(si.on_wait)
                    for w in waits[:-1]:
                        nop = mybir.InstNoOp(
                            name=nc.get_next_instruction_name(),
                            ins=[],
                            outs=[],
                            bass_is_fusable=False,
                        )
                        nop.engine = inst.engine
                        nop.sync_info = mybir.SyncInfo(on_wait=[w], on_update=[])
                        nc.register_instruction(nop, overwrite=True)
                        out.append(nop)
                        n_split += 1
                    inst.sync_info = mybir.SyncInfo(
                        on_wait=[waits[-1]], on_update=list(si.on_update)
                    )
                out.append(inst)
            ordered[bbname] = out
        return ordered

    def postorder_and_split(ordered, start_bb, postordered):
        nc = _CUR_NC[0]
        _split_multi_waits(ordered, nc)
        return _orig_postorder(ordered, start_bb, postordered)

    tile_mod.postorder_instruction_blocks = postorder_and_split

    def _drain_and_barrier_split(self, tick_clock, wait_clock):
        nc = self.nc
        drain_inst = nc.sync.drain()
        wait_clock.add_sem_waits(
            drain_inst.ins, ScopedClock({None: tick_clock.global_clock})
        )
        si = drain_inst.ins.sync_info
        waits = list(si.on_wait)
        if len(waits) > 1:
            drain_inst.ins.sync_info = mybir.SyncInfo(
                on_wait=[waits[0]], on_update=list(si.on_update)
            )
            for w in waits[1:]:
                nop = nc.sync.nop(nofuse=True)
                nop.ins.sync_info = mybir.SyncInfo(on_wait=[w], on_update=[])
        nc.all_engine_barrier()
        assert self.sems is not None
        popped = nc._tile_sem_poison_stack.pop()
        assert popped is self._sem_poison
        nc.clear_and_free_semaphores(list(self.sems.allocated().values()))
        nc.all_engine_barrier()

    tile_mod.TileContext._drain_and_barrier = _drain_and_barrier_split


# ---------------------------------------------------------------- builder
def build_bass(bc=BC):
    """Emit the per-core kernel for `bc` batch items. Returns nc."""
    _patch_tile_drain()
    nc = bass.Bass()
    _CUR_NC[0] = nc

    xt_d = nc.declare_dram_parameter("xt", [DC, 128, bc, S], BF16, isOutput=False)
    wqk_d = nc.declare_dram_parameter("wqk", [128, M_QK, DC, 128], BF16, isOutput=False)
    wv_d = nc.declare_dram_parameter("wv", [128, DC, D], BF16, isOutput=False)
    wo_d = nc.declare_dram_parameter("wo", [128, FC, D], BF16, isOutput=False)
    bqk_d = nc.declare_dram_parameter("bqk", [128, M_QK], F32, isOutput=False)
    bvbc_d = nc.declare_dram_parameter("bvbc", [128, D], F32, isOutput=False)
    bobc_d = nc.declare_dram_parameter("bobc", [128, D], F32, isOutput=False)
    out_d = nc.declare_dram_parameter("out", [bc, S, D], F32, isOutput=True)

    AF = mybir.ActivationFunctionType
    AX = mybir.AxisListType
    OP = mybir.AluOpType

    with tile_mod.TileContext(nc) as tc:
        with (
            tc.tile_pool(name="singles", bufs=1) as singles,
            tc.tile_pool(name="xt", bufs=3) as xpool,
            tc.tile_pool(name="qk", bufs=2) as qkpool,
            tc.tile_pool(name="v", bufs=2) as vpool,
            tc.tile_pool(name="probs", bufs=10) as ppool,
            tc.tile_pool(name="den", bufs=2) as dpool,
            tc.tile_pool(name="dab", bufs=2) as dabpool,
            tc.tile_pool(name="rd", bufs=2) as rpool,
            tc.tile_pool(name="vszp", bufs=2) as vzpool,
            tc.tile_pool(name="ctxT", bufs=2) as cpool,
            tc.tile_pool(name="ot", bufs=4) as opool,
            tc.tile_pool(name="psA", bufs=1, space="PSUM") as psA,
            tc.tile_pool(name="psB", bufs=1, space="PSUM") as psB,
            tc.tile_pool(name="psC", bufs=1, space="PSUM") as psC,
        ):
            # -------- resident weights / biases
            wqk = singles.tile([128, M_QK, DC, 128], BF16)
            nc.sync.dma_start(out=wqk, in_=wqk_d[:])
            wv = singles.tile([128, DC, D], BF16)
            nc.sync.dma_start(out=wv, in_=wv_d[:])
            wo = singles.tile([128, FC, D], BF16)
            nc.sync.dma_start(out=wo, in_=wo_d[:])
            bqk = singles.tile([128, M_QK], F32)
            nc.sync.dma_start(out=bqk, in_=bqk_d[:])
            bvbc = singles.tile([128, D], F32)
            nc.sync.dma_start(out=bvbc, in_=bvbc_d[:])
            bobc = singles.tile([128, D], F32)
            nc.sync.dma_start(out=bobc, in_=bobc_d[:])

            # ktz: K^T zero-PADDED per head so the scores lhsT is a full
            # 128-partition operand. Two dedicated tiles (item b uses
            # ktz[b%2]); the pad halves are zeroed ONCE here and never
            # rewritten (K evictions only touch their own half).
            ktz = [singles.tile([128, FC, 2, S], BF16, name=f"ktz{i}")
                   for i in range(2)]
            for kt in ktz:
                nc.vector.memset(kt[64:128, :, 0, :], 0.0)
                nc.vector.memset(kt[0:64, :, 1, :], 0.0)

            # prime the exp table-set load before the pipeline starts
            warm = singles.tile([128, 2], F32)
            nc.vector.memset(warm[:, 0:1], 0.0)
            nc.scalar.activation(warm[:, 1:2], warm[:, 0:1], AF.Exp)

            # -------- per-item SBUF tile handles
            xts, qks, vs, ctxTs = {}, {}, {}, {}

            def load_xt(b):
                xt = xpool.tile([128, DC, S], BF16, tag="xt")
                for dc in range(DC):
                    nc.sync.dma_start(out=xt[:, dc, :], in_=xt_d[dc, :, b, :])
                xts[b] = xt

            def alloc_item(b):
                qks[b] = qkpool.tile([128, FC, S], BF16, tag="qk", name="qk")
                vs[b] = vpool.tile([128, TT, D], BF16, tag="v", name="v")
                ctxTs[b] = cpool.tile([128, FC, S], BF16, tag="ctx", name="ctx")

            # -------- projection generators (yield per-matmul cycle cost)
            def gen_qkproj(b):
                # NOTE: a matmul with start=True invalidates its ENTIRE
                # 2KB PSUM bank, so every accumulation region must own its
                # banks exclusively -> one [128,768] (2-bank) tile per chunk.
                xt, qk, ktzb = xts[b], qks[b], ktz[b % 2]
                for m in range(FC):
                    ps = psB.tile([128, 768], F32, tag="psB")
                    for dc in range(DC):
                        st, sp = dc == 0, dc == DC - 1
                        for (r0, r1) in ((0, 512), (512, 577)):
                            nc.tensor.matmul(
                                ps[:, r0:r1], lhsT=wqk[:, m, dc, :],
                                rhs=xt[:, dc, r0:r1], start=st, stop=sp)
                            yield r1 - r0
                    # evict Q on DVE (per-partition bias + bf16 cast);
                    # ScalarE is reserved for the Exp critical chain
                    nc.vector.tensor_scalar_add(
                        qk[:, m, :], ps[:, 0:S], bqk[:, m : m + 1])
                    yield 0
                    ps = psB.tile([128, 768], F32, tag="psB")
                    for dc in range(DC):
                        st, sp = dc == 0, dc == DC - 1
                        for (r0, r1) in ((0, 512), (512, 577)):
                            nc.tensor.matmul(
                                ps[:, r0:r1], lhsT=wqk[:, FC + m, dc, :],
                                rhs=xt[:, dc, r0:r1], start=st, stop=sp)
                            yield r1 - r0
                    # evict K halves into the zero-padded layout (DVE)
                    nc.vector.tensor_scalar_add(
                        ktzb[0:64, m, 0, :], ps[0:64, 0:S],
                        bqk[0:64, FC + m : FC + m + 1])
                    nc.vector.tensor_scalar_add(
                        ktzb[64:128, m, 1, :], ps[64:128, 0:S],
                        bqk[64:128, FC + m : FC + m + 1])
                    yield 0

            def gen_tokproj(b, kind):
                # kind 'v': V = xT.T @ Wv ; kind 'o': out = ctxT.T @ Wo
                if kind == "v":
                    lhs_src, rhs_w, nred = xts[b], wv, DC
                else:
                    lhs_src, rhs_w, nred = ctxTs[b], wo, FC
                for tt in range(TT):
                    t0 = tt * 128
                    tsz = min(128, S - t0)
                    ps = psB.tile([128, 768], F32, tag="psB")
                    for rc in range(nred):
                        st, sp = rc == 0, rc == nred - 1
                        for (r0, r1) in ((0, 512), (512, 768)):
                            nc.tensor.matmul(
                                ps[:tsz, r0:r1],
                                lhsT=lhs_src[:, rc, t0 : t0 + tsz],
                                rhs=rhs_w[:, rc, r0:r1], start=st, stop=sp)
                            yield r1 - r0
                    if kind == "v":
                        nc.vector.tensor_add(
                            vs[b][:tsz, tt, :], ps[:tsz, 0:D], bvbc[:tsz])
                    else:
                        ot = opool.tile([128, D], F32, tag="ot")
                        nc.vector.tensor_add(
                            ot[:tsz], ps[:tsz, 0:D], bobc[:tsz])
                        nc.sync.dma_start(
                            out=out_d[b, t0 : t0 + tsz, :], in_=ot[:tsz])
                    yield 0

            proj_gens = deque()

            def drain_proj(quota):
                cy = 0
                while proj_gens and cy < quota:
                    try:
                        cy += next(proj_gens[0])
                    except StopIteration:
                        proj_gens.popleft()
                return cy

            # -------- attention slot machinery
            # Slot (pair p, key-chunk kc): 4 scores MMs -> paired Exp
            # (accum gives denA+denB on ScalarE) -> one denA reduce (DVE).
            # Pair-end: ONE sub (denB for all 5 chunks), ONE reciprocal,
            # and TWO GpSimd tensor_muls that scale the pair's V columns
            # by 1/den (rd broadcast via stride-0 AP). The ctx matmuls run
            # TT+3 slots behind; each head writes its own 64-partition
            # half of the ctx PSUM tile, so no zero-padding of V at all.
            mm_queue = deque()
            psc_box = [None]
            pair_box = [None]
            slot_idx = [0]

            def make_ctx_task(b, p, kc, probs, vszP, ksz):
                def emit():
                    if kc == 0:
                        psc_box[0] = psC.tile([128, S], F32, tag="psC", name="psc")
                    psc = psc_box[0]
                    st, sp = kc == 0, kc == TT - 1
                    for hh in (0, 1):
                        po = hh * 64
                        for (r0, r1) in ((0, 512), (512, 577)):
                            nc.tensor.matmul(
                                psc[po : po + 64, r0:r1],
                                lhsT=vszP[:ksz, kc, po : po + 64],
                                rhs=probs[:ksz, hh, r0:r1], start=st, stop=sp)
                    if kc == TT - 1:
                        nc.vector.tensor_copy(ctxTs[b][:, p, :], psc[:, 0:S])
                return emit

            def emit_slot(b, p, kc, quota):
                g = slot_idx[0]
                slot_idx[0] += 1
                ksz = min(128, S - kc * 128)
                k0 = kc * 128
                qkb, ktzb, vb = qks[b], ktz[b % 2], vs[b]
                # scores pair tile: head A at bank 0, head B at bank 2 (a
                # start=True matmul invalidates its whole 2KB bank, so the
                # two heads' regions must be bank-disjoint)
                ps = psA.tile([128, 2, 1024], F32, tag="psA")
                for (hh, r0, r1) in (
                    (0, 0, 512), (0, 512, 577), (1, 0, 512), (1, 512, 577),
                ):
                    nc.tensor.matmul(
                        ps[:ksz, hh, r0:r1],
                        lhsT=ktzb[:, p, hh, k0 : k0 + ksz],
                        rhs=qkb[:, p, r0:r1], start=True, stop=True)
                if kc == 0:
                    den = dpool.tile([128, TT, 2], F32, tag="den")
                    dab = dabpool.tile([128, TT], F32, tag="dab")
                    rdp = rpool.tile([128, TT, 2], F32, tag="rd")
                    pair_box[0] = (den, dab, rdp)
                den, dab, rdp = pair_box[0]
                # one Exp over both heads; accumulator gives denA+denB
                probs = ppool.tile([128, 2, S], BF16, tag="probs")
                nc.scalar.activation(
                    probs[:ksz, :, :], ps[:ksz, :, 0:S], AF.Exp,
                    accum_out=dab[:ksz, kc : kc + 1])
                # lagged ctx matmuls (from completed pairs)
                while mm_queue and mm_queue[0][0] <= g - (TT + 3):
                    mm_queue.popleft()[1]()
                # proj evictions must precede the Exp-dependent reduce in
                # the in-order DVE stream (they gate single-buffered psB)
                drain_proj(quota)
                nc.vector.tensor_reduce(
                    den[:ksz, kc, 0:1], probs[:ksz, 0, :], axis=AX.X, op=OP.add)
                if kc == TT - 1:
                    nc.vector.tensor_sub(
                        den[:, :, 1], dab[:, :], den[:, :, 0])
                    nc.vector.reciprocal(rdp[:, :, :], den[:, :, :])
                    vszP = vzpool.tile([128, TT, 128], BF16, tag="vszP")
                    c0 = 2 * p * DH
                    nc.gpsimd.tensor_mul(
                        vszP[:, :, 0:64], vb[:, :, c0 : c0 + DH],
                        rdp[:, :, 0:1].broadcast_to((128, TT, DH)))
                    nc.gpsimd.tensor_mul(
                        vszP[:, :, 64:128], vb[:, :, c0 + DH : c0 + 2 * DH],
                        rdp[:, :, 1:2].broadcast_to((128, TT, DH)))
                    pair_box[1:] = [(probs, vszP)]
                pair_probs[kc] = (probs, ksz)
                if kc == TT - 1:
                    vszP = pair_box[1][1]
                    for k2 in range(TT):
                        pr, ks2 = pair_probs[k2]
                        mm_queue.append(
                            (g - (TT - 1) + k2,
                             make_ctx_task(b, p, k2, pr, vszP, ks2)))

            pair_probs = {}

            # -------- prologue: item 0 projections run un-overlapped
            load_xt(0)
            if bc > 1:
                load_xt(1)
            alloc_item(0)
            proj_gens.append(gen_qkproj(0))
            proj_gens.append(gen_tokproj(0, "v"))
            drain_proj(1 << 30)

            # -------- main pipeline
            for b in range(bc):
                # flush item b-1's remaining ctx tasks BEFORE pushing the
                # out-projection that reads ctxT[b-1]: a reader emitted
                # before its writer cannot be ordered by the dep tracker
                while mm_queue:
                    mm_queue.popleft()[1]()
                if b + 2 < bc:
                    load_xt(b + 2)
                if b + 1 < bc:
                    alloc_item(b + 1)
                total = 0
                if b > 0:
                    proj_gens.append(gen_tokproj(b - 1, "o"))
                    total += FC * TT * D  # out-proj: 23040 cy
                if b + 1 < bc:
                    proj_gens.append(gen_qkproj(b + 1))
                    proj_gens.append(gen_tokproj(b + 1, "v"))
                    total += M_QK * DC * S + DC * TT * D  # 41544 + 23040
                quota = total // (NP * TT) + 40
                for p in range(NP):
                    for kc in range(TT):
                        emit_slot(b, p, kc, quota)
                drain_proj(1 << 30)

            # -------- epilogue
            while mm_queue:
                mm_queue.popleft()[1]()
            proj_gens.append(gen_tokproj(bc - 1, "o"))
            drain_proj(1 << 30)

    return nc


# ---------------------------------------------------------------- host prep
def _prep_shared(Wq, bq, Wk, bk, Wv, bv, Wo, bo):
    """Build the per-core-identical weight operands."""
    scale = np.float32(1.0 / np.sqrt(DH))
    wqf = (Wq.astype(np.float32) * scale).transpose(1, 0, 2).reshape(D, D)
    wkf = Wk.astype(np.float32).transpose(1, 0, 2).reshape(D, D)
    wvf = Wv.astype(np.float32).transpose(1, 0, 2).reshape(D, D)

    def chunk4(wf):  # [d, f] -> [di, m, dc, fi]
        return wf.reshape(DC, 128, FC, 128).transpose(1, 2, 0, 3)

    wqk = np.concatenate([chunk4(wqf), chunk4(wkf)], axis=1)  # [128, 12, 6, 128]
    wv3 = wvf.reshape(DC, 128, D).transpose(1, 0, 2)          # [128, 6, 768]
    wo3 = Wo.astype(np.float32).reshape(FC, 128, D).transpose(1, 0, 2)

    bqf = (bq.astype(np.float32) * scale).reshape(D)
    bkf = bk.astype(np.float32).reshape(D)
    bqk = np.concatenate(
        [bqf.reshape(FC, 128), bkf.reshape(FC, 128)], axis=0
    ).T.copy()                                                # [128, 12]
    bvbc = np.broadcast_to(bv.astype(np.float32).reshape(D), (128, D)).copy()
    bobc = np.broadcast_to(bo.astype(np.float32).reshape(D), (128, D)).copy()

    return {
        "wqk": np.ascontiguousarray(wqk).astype(nbf),
        "wv": np.ascontiguousarray(wv3).astype(nbf),
        "wo": np.ascontiguousarray(wo3).astype(nbf),
        "bqk": np.ascontiguousarray(bqk),
        "bvbc": bvbc,
        "bobc": bobc,
    }


_NC_CACHE = {}


def kernel(x, Wq, bq, Wk, bk, Wv, bv, Wo, bo):
    x = np.asarray(x, dtype=np.float32)
    shared = _prep_shared(
        np.asarray(Wq), np.asarray(bq), np.asarray(Wk), np.asarray(bk),
        np.asarray(Wv), np.asarray(bv), np.asarray(Wo), np.asarray(bo))

    in_maps = []
    for c in range(NCORES):
        xc = x[c * BC : (c + 1) * BC]                    # [BC, S, D]
        xt = xc.transpose(2, 0, 1)                       # [D, BC, S]
        xt = xt.reshape(DC, 128, BC, S).astype(nbf)
        m = dict(shared)
        m["xt"] = np.ascontiguousarray(xt)
        in_maps.append(m)

    if "nc" not in _NC_CACHE:
        _NC_CACHE["nc"] = build_bass()
    nc = _NC_CACHE["nc"]

    res = run_bass_kernel_spmd(nc, in_maps, core_ids=list(range(NCORES)))
    out = np.concatenate([res.results[c]["out"] for c in range(NCORES)], axis=0)
    return out.astype(np.float32)


if __name__ == "__main__":
    rng = np.random.default_rng(0)
    ins = {
        "x": rng.standard_normal((B, S, D), dtype=np.float32),
        "Wq": rng.standard_normal((H, D, DH), dtype=np.float32) * 0.02,
        "bq": np.zeros((H, DH), np.float32),
        "Wk": rng.standard_normal((H, D, DH), dtype=np.float32) * 0.02,
        "bk": np.zeros((H, DH), np.float32),
        "Wv": rng.standard_normal((H, D, DH), dtype=np.float32) * 0.02,
        "bv": np.zeros((H, DH), np.float32),
        "Wo": rng.standard_normal((D, D), dtype=np.float32) * 0.02,
        "bo": np.zeros((D,), np.float32),
    }
    o = kernel(**ins)
    print("out", o.shape, o.dtype, float(np.abs(o).max()))


# revision 26
# speedup vs baseline: 1.1940x; 1.0422x over previous
"""Trainium2 Bass kernel for nn_MultiHeadAttention_31542239822105.

Math (faithful to reference, incl. softmax over the QUERY axis):
  q = einsum('bsd,hde->bhse', x, Wq) + bq ; same k, v
  scores = q @ k^T * 1/sqrt(DH)          [B,H,Sq,Sk]
  probs  = softmax(scores, axis=2)       # over q (query axis!)
  ctx    = einsum('bhqk,bhke->bhqe', probs, v)
  out    = ctx.reshape(B,S,D) @ Wo + bo
Sharding: data-parallel over batch, 8 cores x 8 batch items. No collectives.

Per-core layout (all matmul contraction dims land on partitions):
  - x is pre-transposed on the HOST to xT [D, tokens].
  - Q^T,K^T projections come out f-major; V token-major.
  - scoresT[k,q] per head -> softmax over q is a FREE-axis reduction.
  - 1/denominator folded into V rows; ctxT accumulates f-major per head
    pair; output projection emits token-major, direct DMA out.
  - 1/sqrt(DH) folded into Wq/bq on the host.

Schedule: one globally software-pipelined instruction stream built from
"slots", one slot per (head-pair, key-chunk) scores tile:
  - PE: 4 scores matmuls into a PSUM pair tile [128, 2*577] (head pair
    side by side), + ctx matmuls of the slot 2 back, + a cycle-quota of
    projection matmuls for the NEXT batch item (and the output projection
    of the PREVIOUS item). The PE stream never waits on softmax: it ramps
    to the 2.4GHz p-state and stays there.
  - Scalar: ONE Exp activation per slot over the 1154-wide pair tile
    (amortizes the 352-cycle ACT startup), + Q-proj evictions (bias via
    per-partition ACT bias) + ctxT evictions (Copy).
  - DVE: per-slot denominator tensor_reduce [128,2,577]->[128,2] on bf16
    probs, reciprocal, K/V/out-proj evictions.
  - GpSimd (otherwise idle): folds 1/den into the V rows (SBUF->SBUF).
PSUM: scores pair pool 3 banks + proj pair pool 3 banks + ctx 2 banks = 8.
"""

import sys

if "/opt/trn_rl_repo" not in sys.path:
    sys.path.insert(0, "/opt/trn_rl_repo")

from collections import deque

import numpy as np
import ml_dtypes

import concourse.bass as bass
import concourse.mybir as mybir
import concourse.tile as tile_mod
from concourse.vector_clock import ScopedClock
from concourse.bass_utils import run_bass_kernel_spmd

# ---------------------------------------------------------------- constants
B, S, D, H = 64, 577, 768, 12
DH = D // H          # 64
NCORES = 8
BC = B // NCORES     # 8 batch items per core
DC = D // 128        # 6 d-chunks
FC = D // 128        # 6 f-chunks per projection matrix
M_QK = 2 * FC        # 12 combined Q+K f-chunks
TT = (S + 127) // 128  # 5 token tiles (128,128,128,128,65)
NP = H // 2          # 6 head pairs
S2 = 2 * S           # 1154: scores pair tile width

BF16 = mybir.dt.bfloat16
F32 = mybir.dt.float32
nbf = ml_dtypes.bfloat16

_TILE_PATCHED = False
_CUR_NC = [None]


def _patch_tile_drain():
    """The walrus build here rejects >1 sync-wait per instruction
    ("Too many sync wait commands"). Two patches:
    1. post-legalize pass that moves extra waits onto single-wait nops
       inserted just before the offending instruction (same engine);
    2. the final SP Drain (emitted after legalize) gets the same split.
    """
    global _TILE_PATCHED
    if _TILE_PATCHED:
        return
    _TILE_PATCHED = True

    _orig_postorder = tile_mod.postorder_instruction_blocks

    def _split_multi_waits(ordered, nc):
        for bbname, insts in ordered.items():
            out = []
            n_split = 0
            for inst in insts:
                si = inst.sync_info
                if si is not None and len(si.on_wait) > 1:
                    waits = list(si.on_wait)
                    for w in waits[:-1]:
                        nop = mybir.InstNoOp(
                            name=nc.get_next_instruction_name(),
                            ins=[],
                            outs=[],
                            bass_is_fusable=False,
                        )
                        nop.engine = inst.engine
                        nop.sync_info = mybir.SyncInfo(on_wait=[w], on_update=[])
                        nc.register_instruction(nop, overwrite=True)
                        out.append(nop)
                        n_split += 1
                    inst.sync_info = mybir.SyncInfo(
                        on_wait=[waits[-1]], on_update=list(si.on_update)
                    )
                out.append(inst)
            ordered[bbname] = out
        return ordered

    def postorder_and_split(ordered, start_bb, postordered):
        nc = _CUR_NC[0]
        _split_multi_waits(ordered, nc)
        return _orig_postorder(ordered, start_bb, postordered)

    tile_mod.postorder_instruction_blocks = postorder_and_split

    def _drain_and_barrier_split(self, tick_clock, wait_clock):
        nc = self.nc
        drain_inst = nc.sync.drain()
        wait_clock.add_sem_waits(
            drain_inst.ins, ScopedClock({None: tick_clock.global_clock})
        )
        si = drain_inst.ins.sync_info
        waits = list(si.on_wait)
        if len(waits) > 1:
            drain_inst.ins.sync_info = mybir.SyncInfo(
                on_wait=[waits[0]], on_update=list(si.on_update)
            )
            for w in waits[1:]:
                nop = nc.sync.nop(nofuse=True)
                nop.ins.sync_info = mybir.SyncInfo(on_wait=[w], on_update=[])
        nc.all_engine_barrier()
        assert self.sems is not None
        popped = nc._tile_sem_poison_stack.pop()
        assert popped is self._sem_poison
        nc.clear_and_free_semaphores(list(self.sems.allocated().values()))
        nc.all_engine_barrier()

    tile_mod.TileContext._drain_and_barrier = _drain_and_barrier_split


# ---------------------------------------------------------------- builder
def build_bass(bc=BC):
    """Emit the per-core kernel for `bc` batch items. Returns nc."""
    _patch_tile_drain()
    nc = bass.Bass()
    _CUR_NC[0] = nc

    xt_d = nc.declare_dram_parameter("xt", [DC, 128, bc, S], BF16, isOutput=False)
    wqk_d = nc.declare_dram_parameter("wqk", [128, M_QK, DC, 128], BF16, isOutput=False)
    wv_d = nc.declare_dram_parameter("wv", [128, DC, D], BF16, isOutput=False)
    wo_d = nc.declare_dram_parameter("wo", [128, FC, D], BF16, isOutput=False)
    bqk_d = nc.declare_dram_parameter("bqk", [128, M_QK], F32, isOutput=False)
    bvbc_d = nc.declare_dram_parameter("bvbc", [128, D], F32, isOutput=False)
    bobc_d = nc.declare_dram_parameter("bobc", [128, D], F32, isOutput=False)
    out_d = nc.declare_dram_parameter("out", [bc, S, D], F32, isOutput=True)

    AF = mybir.ActivationFunctionType
    AX = mybir.AxisListType
    OP = mybir.AluOpType

    with tile_mod.TileContext(nc) as tc:
        with (
            tc.tile_pool(name="singles", bufs=1) as singles,
            tc.tile_pool(name="xt", bufs=3) as xpool,
            tc.tile_pool(name="qk", bufs=2) as qkpool,
            tc.tile_pool(name="v", bufs=2) as vpool,
            tc.tile_pool(name="probs", bufs=10) as ppool,
            tc.tile_pool(name="den", bufs=2) as dpool,
            tc.tile_pool(name="dab", bufs=2) as dabpool,
            tc.tile_pool(name="rd", bufs=2) as rpool,
            tc.tile_pool(name="vszp", bufs=2) as vzpool,
            tc.tile_pool(name="ctxT", bufs=2) as cpool,
            tc.tile_pool(name="ot", bufs=4) as opool,
            tc.tile_pool(name="psA", bufs=1, space="PSUM") as psA,
            tc.tile_pool(name="psB", bufs=1, space="PSUM") as psB,
            tc.tile_pool(name="psC", bufs=1, space="PSUM") as psC,
        ):
            # -------- resident weights / biases
            wqk = singles.tile([128, M_QK, DC, 128], BF16)
            nc.sync.dma_start(out=wqk, in_=wqk_d[:])
            wv = singles.tile([128, DC, D], BF16)
            nc.sync.dma_start(out=wv, in_=wv_d[:])
            wo = singles.tile([128, FC, D], BF16)
            nc.sync.dma_start(out=wo, in_=wo_d[:])
            bqk = singles.tile([128, M_QK], F32)
            nc.sync.dma_start(out=bqk, in_=bqk_d[:])
            bvbc = singles.tile([128, D], F32)
            nc.sync.dma_start(out=bvbc, in_=bvbc_d[:])
            bobc = singles.tile([128, D], F32)
            nc.sync.dma_start(out=bobc, in_=bobc_d[:])

            # ktz: K^T zero-PADDED per head so the scores lhsT is a full
            # 128-partition operand. Two dedicated tiles (item b uses
            # ktz[b%2]); the pad halves are zeroed ONCE here and never
            # rewritten (K evictions only touch their own half).
            ktz = [singles.tile([128, FC, 2, S], BF16, name=f"ktz{i}")
                   for i in range(2)]
            for kt in ktz:
                nc.vector.memset(kt[64:128, :, 0, :], 0.0)
                nc.vector.memset(kt[0:64, :, 1, :], 0.0)

            # prime the exp table-set load before the pipeline starts
            warm = singles.tile([128, 2], F32)
            nc.vector.memset(warm[:, 0:1], 0.0)
            nc.scalar.activation(warm[:, 1:2], warm[:, 0:1], AF.Exp)

            # -------- per-item SBUF tile handles
            xts, qks, vs, ctxTs = {}, {}, {}, {}

            def load_xt(b):
                xt = xpool.tile([128, DC, S], BF16, tag="xt")
                for dc in range(DC):
                    nc.sync.dma_start(out=xt[:, dc, :], in_=xt_d[dc, :, b, :])
                xts[b] = xt

            def alloc_item(b):
                qks[b] = qkpool.tile([128, FC, S], BF16, tag="qk", name="qk")
                vs[b] = vpool.tile([128, TT, D], BF16, tag="v", name="v")
                ctxTs[b] = cpool.tile([128, FC, S], BF16, tag="ctx", name="ctx")

            # -------- projection generators (yield per-matmul cycle cost)
            def gen_qkproj(b):
                # NOTE: a matmul with start=True invalidates its ENTIRE
                # 2KB PSUM bank, so every accumulation region must own its
                # banks exclusively -> one [128,768] (2-bank) tile per chunk.
                xt, qk, ktzb = xts[b], qks[b], ktz[b % 2]
                for m in range(FC):
                    ps = psB.tile([128, 768], F32, tag="psB")
                    for dc in range(DC):
                        st, sp = dc == 0, dc == DC - 1
                        for (r0, r1) in ((0, 512), (512, 577)):
                            nc.tensor.matmul(
                                ps[:, r0:r1], lhsT=wqk[:, m, dc, :],
                                rhs=xt[:, dc, r0:r1], start=st, stop=sp)
                            yield r1 - r0
                    # evict Q on DVE (per-partition bias + bf16 cast);
                    # ScalarE is reserved for the Exp critical chain
                    nc.vector.tensor_scalar_add(
                        qk[:, m, :], ps[:, 0:S], bqk[:, m : m + 1])
                    yield 0
                    ps = psB.tile([128, 768], F32, tag="psB")
                    for dc in range(DC):
                        st, sp = dc == 0, dc == DC - 1
                        for (r0, r1) in ((0, 512), (512, 577)):
                            nc.tensor.matmul(
                                ps[:, r0:r1], lhsT=wqk[:, FC + m, dc, :],
                                rhs=xt[:, dc, r0:r1], start=st, stop=sp)
                            yield r1 - r0
                    # evict K halves into the zero-padded layout (DVE)
                    nc.vector.tensor_scalar_add(
                        ktzb[0:64, m, 0, :], ps[0:64, 0:S],
                        bqk[0:64, FC + m : FC + m + 1])
                    nc.vector.tensor_scalar_add(
                        ktzb[64:128, m, 1, :], ps[64:128, 0:S],
                        bqk[64:128, FC + m : FC + m + 1])
                    yield 0

            def gen_tokproj(b, kind):
                # kind 'v': V = xT.T @ Wv ; kind 'o': out = ctxT.T @ Wo
                if kind == "v":
                    lhs_src, rhs_w, nred = xts[b], wv, DC
                else:
                    lhs_src, rhs_w, nred = ctxTs[b], wo, FC
                for tt in range(TT):
                    t0 = tt * 128
                    tsz = min(128, S - t0)
                    ps = psB.tile([128, 768], F32, tag="psB")
                    for rc in range(nred):
                        st, sp = rc == 0, rc == nred - 1
                        for (r0, r1) in ((0, 512), (512, 768)):
                            nc.tensor.matmul(
                                ps[:tsz, r0:r1],
                                lhsT=lhs_src[:, rc, t0 : t0 + tsz],
                                rhs=rhs_w[:, rc, r0:r1], start=st, stop=sp)
                            yield r1 - r0
                    if kind == "v":
                        nc.vector.tensor_add(
                            vs[b][:tsz, tt, :], ps[:tsz, 0:D], bvbc[:tsz])
                    else:
                        ot = opool.tile([128, D], F32, tag="ot")
                        nc.vector.tensor_add(
                            ot[:tsz], ps[:tsz, 0:D], bobc[:tsz])
                        nc.sync.dma_start(
                            out=out_d[b, t0 : t0 + tsz, :], in_=ot[:tsz])
                    yield 0

            proj_gens = deque()

            def drain_proj(quota):
                cy = 0
                while proj_gens and cy < quota:
                    try:
                        cy += next(proj_gens[0])
                    except StopIteration:
                        proj_gens.popleft()
                return cy

            # -------- attention slot machinery
            # Slot (pair p, key-chunk kc): 4 scores MMs -> paired Exp
            # (accum gives denA+denB on ScalarE) -> one denA reduce (DVE).
            # Pair-end: ONE sub (denB for all 5 chunks), ONE reciprocal,
            # and TWO GpSimd tensor_muls that scale the pair's V columns
            # by 1/den (rd broadcast via stride-0 AP). The ctx matmuls run
            # TT+3 slots behind; each head writes its own 64-partition
            # half of the ctx PSUM tile, so no zero-padding of V at all.
            mm_queue = deque()
            psc_box = [None]
            pair_box = [None]
            slot_idx = [0]

            def make_ctx_task(b, p, kc, probs, vszP, ksz):
                def emit():
                    if kc == 0:
                        psc_box[0] = psC.tile([128, S], F32, tag="psC", name="psc")
                    psc = psc_box[0]
                    st, sp = kc == 0, kc == TT - 1
                    for hh in (0, 1):
                        po = hh * 64
                        for (r0, r1) in ((0, 512), (512, 577)):
                            nc.tensor.matmul(
                                psc[po : po + 64, r0:r1],
                                lhsT=vszP[:ksz, kc, po : po + 64],
                                rhs=probs[:ksz, hh, r0:r1], start=st, stop=sp)
                    if kc == TT - 1:
                        nc.vector.tensor_copy(ctxTs[b][:, p, :], psc[:, 0:S])
                return emit

            def emit_slot(b, p, kc, quota):
                g = slot_idx[0]
                slot_idx[0] += 1
                ksz = min(128, S - kc * 128)
                k0 = kc * 128
                qkb, ktzb, vb = qks[b], ktz[b % 2], vs[b]
                # scores pair tile: head A at bank 0, head B at bank 2 (a
                # start=True matmul invalidates its whole 2KB bank, so the
                # two heads' regions must be bank-disjoint)
                ps = psA.tile([128, 2, 1024], F32, tag="psA")
                for (hh, r0, r1) in (
                    (0, 0, 512), (0, 512, 577), (1, 0, 512), (1, 512, 577),
                ):
                    nc.tensor.matmul(
                        ps[:ksz, hh, r0:r1],
                        lhsT=ktzb[:, p, hh, k0 : k0 + ksz],
                        rhs=qkb[:, p, r0:r1], start=True, stop=True)
                if kc == 0:
                    den = dpool.tile([128, TT, 2], F32, tag="den")
                    dab = dabpool.tile([128, TT], F32, tag="dab")
                    rdp = rpool.tile([128, TT, 2], F32, tag="rd")
                    pair_box[0] = (den, dab, rdp)
                den, dab, rdp = pair_box[0]
                # one Exp over both heads; accumulator gives denA+denB
                probs = ppool.tile([128, 2, S], BF16, tag="probs")
                nc.scalar.activation(
                    probs[:ksz, :, :], ps[:ksz, :, 0:S], AF.Exp,
                    accum_out=dab[:ksz, kc : kc + 1])
                # lagged ctx matmuls (from completed pairs)
                while mm_queue and mm_queue[0][0] <= g - (TT + 3):
                    mm_queue.popleft()[1]()
                # proj evictions must precede the Exp-dependent reduce in
                # the in-order DVE stream (they gate single-buffered psB)
                drain_proj(quota)
                nc.vector.tensor_reduce(
                    den[:ksz, kc, 0:1], probs[:ksz, 0, :], axis=AX.X, op=OP.add)
                if kc == TT - 1:
                    nc.vector.tensor_sub(
                        den[:, :, 1], dab[:, :], den[:, :, 0])
                    nc.vector.reciprocal(rdp[:, :, :], den[:, :, :])
                    vszP = vzpool.tile([128, TT, 128], BF16, tag="vszP")
                    c0 = 2 * p * DH
                    nc.gpsimd.tensor_mul(
                        vszP[:, :, 0:64], vb[:, :, c0 : c0 + DH],
                        rdp[:, :, 0:1].broadcast_to((128, TT, DH)))
                    nc.gpsimd.tensor_mul(
                        vszP[:, :, 64:128], vb[:, :, c0 + DH : c0 + 2 * DH],
                        rdp[:, :, 1:2].broadcast_to((128, TT, DH)))
                    pair_box[1:] = [(probs, vszP)]
                pair_probs[kc] = (probs, ksz)
                if kc == TT - 1:
                    vszP = pair_box[1][1]
                    for k2 in range(TT):
                        pr, ks2 = pair_probs[k2]
                        mm_queue.append(
                            (g - (TT - 1) + k2,
                             make_ctx_task(b, p, k2, pr, vszP, ks2)))

            pair_probs = {}

            # -------- prologue: item 0 projections run un-overlapped
            load_xt(0)
            if bc > 1:
                load_xt(1)
            alloc_item(0)
            proj_gens.append(gen_qkproj(0))
            proj_gens.append(gen_tokproj(0, "v"))
            drain_proj(1 << 30)

            # -------- main pipeline
            for b in range(bc):
                # flush item b-1's remaining ctx tasks BEFORE pushing the
                # out-projection that reads ctxT[b-1]: a reader emitted
                # before its writer cannot be ordered by the dep tracker
                while mm_queue:
                    mm_queue.popleft()[1]()
                if b + 2 < bc:
                    load_xt(b + 2)
                if b + 1 < bc:
                    alloc_item(b + 1)
                total = 0
                if b + 1 < bc:
                    # next item's projections FIRST: always dispatchable
                    # (xt loaded an item ahead); the out-projection of the
                    # previous item LAST — its first matmuls chain on the
                    # freshly flushed ctx evictions and would stall the PE
                    proj_gens.append(gen_qkproj(b + 1))
                    proj_gens.append(gen_tokproj(b + 1, "v"))
                    total += M_QK * DC * S + DC * TT * D  # 41544 + 23040
                if b > 0:
                    proj_gens.append(gen_tokproj(b - 1, "o"))
                    total += FC * TT * D  # out-proj: 23040 cy
                quota = -(-total // (NP * TT))
                for p in range(NP):
                    for kc in range(TT):
                        emit_slot(b, p, kc, quota)
                drain_proj(1 << 30)

            # -------- epilogue
            while mm_queue:
                mm_queue.popleft()[1]()
            proj_gens.append(gen_tokproj(bc - 1, "o"))
            drain_proj(1 << 30)

    return nc


# ---------------------------------------------------------------- host prep
def _prep_shared(Wq, bq, Wk, bk, Wv, bv, Wo, bo):
    """Build the per-core-identical weight operands."""
    scale = np.float32(1.0 / np.sqrt(DH))
    wqf = (Wq.astype(np.float32) * scale).transpose(1, 0, 2).reshape(D, D)
    wkf = Wk.astype(np.float32).transpose(1, 0, 2).reshape(D, D)
    wvf = Wv.astype(np.float32).transpose(1, 0, 2).reshape(D, D)

    def chunk4(wf):  # [d, f] -> [di, m, dc, fi]
        return wf.reshape(DC, 128, FC, 128).transpose(1, 2, 0, 3)

    wqk = np.concatenate([chunk4(wqf), chunk4(wkf)], axis=1)  # [128, 12, 6, 128]
    wv3 = wvf.reshape(DC, 128, D).transpose(1, 0, 2)          # [128, 6, 768]
    wo3 = Wo.astype(np.float32).reshape(FC, 128, D).transpose(1, 0, 2)

    bqf = (bq.astype(np.float32) * scale).reshape(D)
    bkf = bk.astype(np.float32).reshape(D)
    bqk = np.concatenate(
        [bqf.reshape(FC, 128), bkf.reshape(FC, 128)], axis=0
    ).T.copy()                                                # [128, 12]
    bvbc = np.broadcast_to(bv.astype(np.float32).reshape(D), (128, D)).copy()
    bobc = np.broadcast_to(bo.astype(np.float32).reshape(D), (128, D)).copy()

    return {
        "wqk": np.ascontiguousarray(wqk).astype(nbf),
        "wv": np.ascontiguousarray(wv3).astype(nbf),
        "wo": np.ascontiguousarray(wo3).astype(nbf),
        "bqk": np.ascontiguousarray(bqk),
        "bvbc": bvbc,
        "bobc": bobc,
    }


_NC_CACHE = {}


def kernel(x, Wq, bq, Wk, bk, Wv, bv, Wo, bo):
    x = np.asarray(x, dtype=np.float32)
    shared = _prep_shared(
        np.asarray(Wq), np.asarray(bq), np.asarray(Wk), np.asarray(bk),
        np.asarray(Wv), np.asarray(bv), np.asarray(Wo), np.asarray(bo))

    in_maps = []
    for c in range(NCORES):
        xc = x[c * BC : (c + 1) * BC]                    # [BC, S, D]
        xt = xc.transpose(2, 0, 1)                       # [D, BC, S]
        xt = xt.reshape(DC, 128, BC, S).astype(nbf)
        m = dict(shared)
        m["xt"] = np.ascontiguousarray(xt)
        in_maps.append(m)

    if "nc" not in _NC_CACHE:
        _NC_CACHE["nc"] = build_bass()
    nc = _NC_CACHE["nc"]

    res = run_bass_kernel_spmd(nc, in_maps, core_ids=list(range(NCORES)))
    out = np.concatenate([res.results[c]["out"] for c in range(NCORES)], axis=0)
    return out.astype(np.float32)


if __name__ == "__main__":
    rng = np.random.default_rng(0)
    ins = {
        "x": rng.standard_normal((B, S, D), dtype=np.float32),
        "Wq": rng.standard_normal((H, D, DH), dtype=np.float32) * 0.02,
        "bq": np.zeros((H, DH), np.float32),
        "Wk": rng.standard_normal((H, D, DH), dtype=np.float32) * 0.02,
        "bk": np.zeros((H, DH), np.float32),
        "Wv": rng.standard_normal((H, D, DH), dtype=np.float32) * 0.02,
        "bv": np.zeros((H, DH), np.float32),
        "Wo": rng.standard_normal((D, D), dtype=np.float32) * 0.02,
        "bo": np.zeros((D,), np.float32),
    }
    o = kernel(**ins)
    print("out", o.shape, o.dtype, float(np.abs(o).max()))


# revision 27
# speedup vs baseline: 1.2071x; 1.0110x over previous
"""Trainium2 Bass kernel for nn_MultiHeadAttention_31542239822105.

Math (faithful to reference, incl. softmax over the QUERY axis):
  q = einsum('bsd,hde->bhse', x, Wq) + bq ; same k, v
  scores = q @ k^T * 1/sqrt(DH)          [B,H,Sq,Sk]
  probs  = softmax(scores, axis=2)       # over q (query axis!)
  ctx    = einsum('bhqk,bhke->bhqe', probs, v)
  out    = ctx.reshape(B,S,D) @ Wo + bo
Sharding: data-parallel over batch, 8 cores x 8 batch items. No collectives.

Per-core layout (all matmul contraction dims land on partitions):
  - x is pre-transposed on the HOST to xT [D, tokens].
  - Q^T,K^T projections come out f-major; V token-major.
  - scoresT[k,q] per head -> softmax over q is a FREE-axis reduction.
  - 1/denominator folded into V rows; ctxT accumulates f-major per head
    pair; output projection emits token-major, direct DMA out.
  - 1/sqrt(DH) folded into Wq/bq on the host.

Schedule: one globally software-pipelined instruction stream built from
"slots", one slot per (head-pair, key-chunk) scores tile:
  - PE: 4 scores matmuls into a PSUM pair tile [128, 2*577] (head pair
    side by side), + ctx matmuls of the slot 2 back, + a cycle-quota of
    projection matmuls for the NEXT batch item (and the output projection
    of the PREVIOUS item). The PE stream never waits on softmax: it ramps
    to the 2.4GHz p-state and stays there.
  - Scalar: ONE Exp activation per slot over the 1154-wide pair tile
    (amortizes the 352-cycle ACT startup), + Q-proj evictions (bias via
    per-partition ACT bias) + ctxT evictions (Copy).
  - DVE: per-slot denominator tensor_reduce [128,2,577]->[128,2] on bf16
    probs, reciprocal, K/V/out-proj evictions.
  - GpSimd (otherwise idle): folds 1/den into the V rows (SBUF->SBUF).
PSUM: scores pair pool 3 banks + proj pair pool 3 banks + ctx 2 banks = 8.
"""

import sys

if "/opt/trn_rl_repo" not in sys.path:
    sys.path.insert(0, "/opt/trn_rl_repo")

from collections import deque

import numpy as np
import ml_dtypes

import concourse.bass as bass
import concourse.mybir as mybir
import concourse.tile as tile_mod
from concourse.vector_clock import ScopedClock
from concourse.bass_utils import run_bass_kernel_spmd

# ---------------------------------------------------------------- constants
B, S, D, H = 64, 577, 768, 12
DH = D // H          # 64
NCORES = 8
BC = B // NCORES     # 8 batch items per core
DC = D // 128        # 6 d-chunks
FC = D // 128        # 6 f-chunks per projection matrix
M_QK = 2 * FC        # 12 combined Q+K f-chunks
TT = (S + 127) // 128  # 5 token tiles (128,128,128,128,65)
NP = H // 2          # 6 head pairs
S2 = 2 * S           # 1154: scores pair tile width

BF16 = mybir.dt.bfloat16
F32 = mybir.dt.float32
nbf = ml_dtypes.bfloat16

_TILE_PATCHED = False
_CUR_NC = [None]


def _patch_tile_drain():
    """The walrus build here rejects >1 sync-wait per instruction
    ("Too many sync wait commands"). Two patches:
    1. post-legalize pass that moves extra waits onto single-wait nops
       inserted just before the offending instruction (same engine);
    2. the final SP Drain (emitted after legalize) gets the same split.
    """
    global _TILE_PATCHED
    if _TILE_PATCHED:
        return
    _TILE_PATCHED = True

    _orig_postorder = tile_mod.postorder_instruction_blocks

    def _split_multi_waits(ordered, nc):
        for bbname, insts in ordered.items():
            out = []
            n_split = 0
            for inst in insts:
                si = inst.sync_info
                if si is not None and len(si.on_wait) > 1:
                    waits = list(si.on_wait)
                    for w in waits[:-1]:
                        nop = mybir.InstNoOp(
                            name=nc.get_next_instruction_name(),
                            ins=[],
                            outs=[],
                            bass_is_fusable=False,
                        )
                        nop.engine = inst.engine
                        nop.sync_info = mybir.SyncInfo(on_wait=[w], on_update=[])
                        nc.register_instruction(nop, overwrite=True)
                        out.append(nop)
                        n_split += 1
                    inst.sync_info = mybir.SyncInfo(
                        on_wait=[waits[-1]], on_update=list(si.on_update)
                    )
                out.append(inst)
            ordered[bbname] = out
        return ordered

    def postorder_and_split(ordered, start_bb, postordered):
        nc = _CUR_NC[0]
        _split_multi_waits(ordered, nc)
        return _orig_postorder(ordered, start_bb, postordered)

    tile_mod.postorder_instruction_blocks = postorder_and_split

    def _drain_and_barrier_split(self, tick_clock, wait_clock):
        nc = self.nc
        drain_inst = nc.sync.drain()
        wait_clock.add_sem_waits(
            drain_inst.ins, ScopedClock({None: tick_clock.global_clock})
        )
        si = drain_inst.ins.sync_info
        waits = list(si.on_wait)
        if len(waits) > 1:
            drain_inst.ins.sync_info = mybir.SyncInfo(
                on_wait=[waits[0]], on_update=list(si.on_update)
            )
            for w in waits[1:]:
                nop = nc.sync.nop(nofuse=True)
                nop.ins.sync_info = mybir.SyncInfo(on_wait=[w], on_update=[])
        nc.all_engine_barrier()
        assert self.sems is not None
        popped = nc._tile_sem_poison_stack.pop()
        assert popped is self._sem_poison
        nc.clear_and_free_semaphores(list(self.sems.allocated().values()))
        nc.all_engine_barrier()

    tile_mod.TileContext._drain_and_barrier = _drain_and_barrier_split


# ---------------------------------------------------------------- builder
def build_bass(bc=BC):
    """Emit the per-core kernel for `bc` batch items. Returns nc."""
    _patch_tile_drain()
    nc = bass.Bass()
    _CUR_NC[0] = nc

    xt_d = nc.declare_dram_parameter("xt", [DC, 128, bc, S], BF16, isOutput=False)
    wqk_d = nc.declare_dram_parameter("wqk", [128, M_QK, DC, 128], BF16, isOutput=False)
    wv_d = nc.declare_dram_parameter("wv", [128, DC, D], BF16, isOutput=False)
    wo_d = nc.declare_dram_parameter("wo", [128, FC, D], BF16, isOutput=False)
    bqk_d = nc.declare_dram_parameter("bqk", [128, M_QK], F32, isOutput=False)
    bvbc_d = nc.declare_dram_parameter("bvbc", [128, D], F32, isOutput=False)
    bobc_d = nc.declare_dram_parameter("bobc", [128, D], F32, isOutput=False)
    out_d = nc.declare_dram_parameter("out", [bc, S, D], F32, isOutput=True)

    AF = mybir.ActivationFunctionType
    AX = mybir.AxisListType
    OP = mybir.AluOpType

    with tile_mod.TileContext(nc) as tc:
        with (
            tc.tile_pool(name="singles", bufs=1) as singles,
            tc.tile_pool(name="xt", bufs=3) as xpool,
            tc.tile_pool(name="qk", bufs=2) as qkpool,
            tc.tile_pool(name="v", bufs=2) as vpool,
            tc.tile_pool(name="probs", bufs=10) as ppool,
            tc.tile_pool(name="den", bufs=2) as dpool,
            tc.tile_pool(name="dab", bufs=2) as dabpool,
            tc.tile_pool(name="rd", bufs=2) as rpool,
            tc.tile_pool(name="vszp", bufs=2) as vzpool,
            tc.tile_pool(name="ctxT", bufs=2) as cpool,
            tc.tile_pool(name="ot", bufs=4) as opool,
            tc.tile_pool(name="psA", bufs=1, space="PSUM") as psA,
            tc.tile_pool(name="psB", bufs=1, space="PSUM") as psB,
            tc.tile_pool(name="psC", bufs=1, space="PSUM") as psC,
        ):
            # -------- resident weights / biases
            wqk = singles.tile([128, M_QK, DC, 128], BF16)
            nc.sync.dma_start(out=wqk, in_=wqk_d[:])
            wv = singles.tile([128, DC, D], BF16)
            nc.sync.dma_start(out=wv, in_=wv_d[:])
            wo = singles.tile([128, FC, D], BF16)
            nc.sync.dma_start(out=wo, in_=wo_d[:])
            bqk = singles.tile([128, M_QK], F32)
            nc.sync.dma_start(out=bqk, in_=bqk_d[:])
            bvbc = singles.tile([128, D], F32)
            nc.sync.dma_start(out=bvbc, in_=bvbc_d[:])
            bobc = singles.tile([128, D], F32)
            nc.sync.dma_start(out=bobc, in_=bobc_d[:])

            # ktz: K^T zero-PADDED per head so the scores lhsT is a full
            # 128-partition operand. Two dedicated tiles (item b uses
            # ktz[b%2]); the pad halves are zeroed ONCE here and never
            # rewritten (K evictions only touch their own half).
            ktz = [singles.tile([128, FC, 2, S], BF16, name=f"ktz{i}")
                   for i in range(2)]
            for kt in ktz:
                nc.vector.memset(kt[64:128, :, 0, :], 0.0)
                nc.vector.memset(kt[0:64, :, 1, :], 0.0)

            # prime the exp table-set load before the pipeline starts
            warm = singles.tile([128, 2], F32)
            nc.vector.memset(warm[:, 0:1], 0.0)
            nc.scalar.activation(warm[:, 1:2], warm[:, 0:1], AF.Exp)

            # -------- per-item SBUF tile handles
            xts, qks, vs, ctxTs = {}, {}, {}, {}

            def load_xt(b):
                xt = xpool.tile([128, DC, S], BF16, tag="xt")
                for dc in range(DC):
                    nc.sync.dma_start(out=xt[:, dc, :], in_=xt_d[dc, :, b, :])
                xts[b] = xt

            def alloc_item(b):
                qks[b] = qkpool.tile([128, FC, S], BF16, tag="qk", name="qk")
                vs[b] = vpool.tile([128, TT, D], BF16, tag="v", name="v")
                ctxTs[b] = cpool.tile([128, FC, S], BF16, tag="ctx", name="ctx")

            # -------- projection generators (yield per-matmul cycle cost)
            def gen_qkproj(b, m0=0, m1=FC):
                # NOTE: a matmul with start=True invalidates its ENTIRE
                # 2KB PSUM bank, so every accumulation region must own its
                # banks exclusively -> one [128,768] (2-bank) tile per chunk.
                xt, qk, ktzb = xts[b], qks[b], ktz[b % 2]
                for m in range(m0, m1):
                    ps = psB.tile([128, 768], F32, tag="psB")
                    for dc in range(DC):
                        st, sp = dc == 0, dc == DC - 1
                        for (r0, r1) in ((0, 512), (512, 577)):
                            nc.tensor.matmul(
                                ps[:, r0:r1], lhsT=wqk[:, m, dc, :],
                                rhs=xt[:, dc, r0:r1], start=st, stop=sp)
                            yield r1 - r0
                    # evict Q on DVE (per-partition bias + bf16 cast);
                    # ScalarE is reserved for the Exp critical chain
                    nc.vector.tensor_scalar_add(
                        qk[:, m, :], ps[:, 0:S], bqk[:, m : m + 1])
                    yield 0
                    ps = psB.tile([128, 768], F32, tag="psB")
                    for dc in range(DC):
                        st, sp = dc == 0, dc == DC - 1
                        for (r0, r1) in ((0, 512), (512, 577)):
                            nc.tensor.matmul(
                                ps[:, r0:r1], lhsT=wqk[:, FC + m, dc, :],
                                rhs=xt[:, dc, r0:r1], start=st, stop=sp)
                            yield r1 - r0
                    # evict K halves into the zero-padded layout (DVE)
                    nc.vector.tensor_scalar_add(
                        ktzb[0:64, m, 0, :], ps[0:64, 0:S],
                        bqk[0:64, FC + m : FC + m + 1])
                    nc.vector.tensor_scalar_add(
                        ktzb[64:128, m, 1, :], ps[64:128, 0:S],
                        bqk[64:128, FC + m : FC + m + 1])
                    yield 0

            def gen_tokproj(b, kind):
                # kind 'v': V = xT.T @ Wv ; kind 'o': out = ctxT.T @ Wo
                if kind == "v":
                    lhs_src, rhs_w, nred = xts[b], wv, DC
                else:
                    lhs_src, rhs_w, nred = ctxTs[b], wo, FC
                for tt in range(TT):
                    t0 = tt * 128
                    tsz = min(128, S - t0)
                    ps = psB.tile([128, 768], F32, tag="psB")
                    for rc in range(nred):
                        st, sp = rc == 0, rc == nred - 1
                        for (r0, r1) in ((0, 512), (512, 768)):
                            nc.tensor.matmul(
                                ps[:tsz, r0:r1],
                                lhsT=lhs_src[:, rc, t0 : t0 + tsz],
                                rhs=rhs_w[:, rc, r0:r1], start=st, stop=sp)
                            yield r1 - r0
                    if kind == "v":
                        nc.vector.tensor_add(
                            vs[b][:tsz, tt, :], ps[:tsz, 0:D], bvbc[:tsz])
                    else:
                        ot = opool.tile([128, D], F32, tag="ot")
                        nc.vector.tensor_add(
                            ot[:tsz], ps[:tsz, 0:D], bobc[:tsz])
                        nc.sync.dma_start(
                            out=out_d[b, t0 : t0 + tsz, :], in_=ot[:tsz])
                    yield 0

            proj_gens = deque()

            def drain_proj(quota):
                cy = 0
                while proj_gens and cy < quota:
                    try:
                        cy += next(proj_gens[0])
                    except StopIteration:
                        proj_gens.popleft()
                return cy

            # -------- attention slot machinery
            # Slot (pair p, key-chunk kc): 4 scores MMs -> paired Exp
            # (accum gives denA+denB on ScalarE) -> one denA reduce (DVE).
            # Pair-end: ONE sub (denB for all 5 chunks), ONE reciprocal,
            # and TWO GpSimd tensor_muls that scale the pair's V columns
            # by 1/den (rd broadcast via stride-0 AP). The ctx matmuls run
            # TT+3 slots behind; each head writes its own 64-partition
            # half of the ctx PSUM tile, so no zero-padding of V at all.
            mm_queue = deque()
            psc_box = [None]
            pair_box = [None]
            slot_idx = [0]

            def make_ctx_task(b, p, kc, probs, vszP, ksz):
                def emit():
                    if kc == 0:
                        psc_box[0] = psC.tile([128, S], F32, tag="psC", name="psc")
                    psc = psc_box[0]
                    st, sp = kc == 0, kc == TT - 1
                    for hh in (0, 1):
                        po = hh * 64
                        for (r0, r1) in ((0, 512), (512, 577)):
                            nc.tensor.matmul(
                                psc[po : po + 64, r0:r1],
                                lhsT=vszP[:ksz, kc, po : po + 64],
                                rhs=probs[:ksz, hh, r0:r1], start=st, stop=sp)
                    if kc == TT - 1:
                        nc.vector.tensor_copy(ctxTs[b][:, p, :], psc[:, 0:S])
                return emit

            def emit_slot(b, p, kc, quota):
                g = slot_idx[0]
                slot_idx[0] += 1
                ksz = min(128, S - kc * 128)
                k0 = kc * 128
                qkb, ktzb, vb = qks[b], ktz[b % 2], vs[b]
                # scores pair tile: head A at bank 0, head B at bank 2 (a
                # start=True matmul invalidates its whole 2KB bank, so the
                # two heads' regions must be bank-disjoint)
                ps = psA.tile([128, 2, 1024], F32, tag="psA")
                for (hh, r0, r1) in (
                    (0, 0, 512), (0, 512, 577), (1, 0, 512), (1, 512, 577),
                ):
                    nc.tensor.matmul(
                        ps[:ksz, hh, r0:r1],
                        lhsT=ktzb[:, p, hh, k0 : k0 + ksz],
                        rhs=qkb[:, p, r0:r1], start=True, stop=True)
                if kc == 0:
                    den = dpool.tile([128, TT, 2], F32, tag="den")
                    dab = dabpool.tile([128, TT], F32, tag="dab")
                    rdp = rpool.tile([128, TT, 2], F32, tag="rd")
                    pair_box[0] = (den, dab, rdp)
                den, dab, rdp = pair_box[0]
                # one Exp over both heads; accumulator gives denA+denB
                probs = ppool.tile([128, 2, S], BF16, tag="probs")
                nc.scalar.activation(
                    probs[:ksz, :, :], ps[:ksz, :, 0:S], AF.Exp,
                    accum_out=dab[:ksz, kc : kc + 1])
                # lagged ctx matmuls (from completed pairs)
                while mm_queue and mm_queue[0][0] <= g - (TT + 3):
                    mm_queue.popleft()[2]()
                # proj evictions must precede the Exp-dependent reduce in
                # the in-order DVE stream (they gate single-buffered psB)
                drain_proj(quota)
                nc.vector.tensor_reduce(
                    den[:ksz, kc, 0:1], probs[:ksz, 0, :], axis=AX.X, op=OP.add)
                if kc == TT - 1:
                    nc.vector.tensor_sub(
                        den[:, :, 1], dab[:, :], den[:, :, 0])
                    nc.vector.reciprocal(rdp[:, :, :], den[:, :, :])
                    vszP = vzpool.tile([128, TT, 128], BF16, tag="vszP")
                    c0 = 2 * p * DH
                    nc.gpsimd.tensor_mul(
                        vszP[:, :, 0:64], vb[:, :, c0 : c0 + DH],
                        rdp[:, :, 0:1].broadcast_to((128, TT, DH)))
                    nc.gpsimd.tensor_mul(
                        vszP[:, :, 64:128], vb[:, :, c0 + DH : c0 + 2 * DH],
                        rdp[:, :, 1:2].broadcast_to((128, TT, DH)))
                    pair_box[1:] = [(probs, vszP)]
                pair_probs[kc] = (probs, ksz)
                if kc == TT - 1:
                    vszP = pair_box[1][1]
                    for k2 in range(TT):
                        pr, ks2 = pair_probs[k2]
                        mm_queue.append(
                            (g - (TT - 1) + k2, b,
                             make_ctx_task(b, p, k2, pr, vszP, ks2)))

            pair_probs = {}

            def gen_flush(upto_b):
                # ctx tasks of items <= upto_b must be emitted before the
                # out-projection that reads their ctxT (a reader emitted
                # before its writer cannot be ordered by the dep tracker)
                while mm_queue and mm_queue[0][1] <= upto_b:
                    mm_queue.popleft()[2]()
                    yield 0

            # -------- prologue: only pair-0 Q/K and V of item 0 run
            # un-overlapped; the rest of QK(0) becomes slot filler
            load_xt(0)
            if bc > 1:
                load_xt(1)
            alloc_item(0)
            proj_gens.append(gen_qkproj(0, 0, 1))
            proj_gens.append(gen_tokproj(0, "v"))
            drain_proj(1 << 30)

            # -------- main pipeline
            for b in range(bc):
                if b + 2 < bc:
                    load_xt(b + 2)
                if b + 1 < bc:
                    alloc_item(b + 1)
                total = 0
                if b == 0:
                    proj_gens.append(gen_qkproj(0, 1, FC))
                    total += (M_QK - 2) * DC * S
                if b + 1 < bc:
                    # next item's projections first: always dispatchable
                    proj_gens.append(gen_qkproj(b + 1))
                    proj_gens.append(gen_tokproj(b + 1, "v"))
                    total += M_QK * DC * S + DC * TT * D  # 41544 + 23040
                if b > 0:
                    proj_gens.append(gen_flush(b - 1))
                    proj_gens.append(gen_tokproj(b - 1, "o"))
                    total += FC * TT * D  # out-proj: 23040 cy
                quota = -(-total // (NP * TT))
                for p in range(NP):
                    for kc in range(TT):
                        emit_slot(b, p, kc, quota)
                drain_proj(1 << 30)

            # -------- epilogue
            while mm_queue:
                mm_queue.popleft()[2]()
            proj_gens.append(gen_tokproj(bc - 1, "o"))
            drain_proj(1 << 30)

    return nc


# ---------------------------------------------------------------- host prep
def _prep_shared(Wq, bq, Wk, bk, Wv, bv, Wo, bo):
    """Build the per-core-identical weight operands."""
    scale = np.float32(1.0 / np.sqrt(DH))
    wqf = (Wq.astype(np.float32) * scale).transpose(1, 0, 2).reshape(D, D)
    wkf = Wk.astype(np.float32).transpose(1, 0, 2).reshape(D, D)
    wvf = Wv.astype(np.float32).transpose(1, 0, 2).reshape(D, D)

    def chunk4(wf):  # [d, f] -> [di, m, dc, fi]
        return wf.reshape(DC, 128, FC, 128).transpose(1, 2, 0, 3)

    wqk = np.concatenate([chunk4(wqf), chunk4(wkf)], axis=1)  # [128, 12, 6, 128]
    wv3 = wvf.reshape(DC, 128, D).transpose(1, 0, 2)          # [128, 6, 768]
    wo3 = Wo.astype(np.float32).reshape(FC, 128, D).transpose(1, 0, 2)

    bqf = (bq.astype(np.float32) * scale).reshape(D)
    bkf = bk.astype(np.float32).reshape(D)
    bqk = np.concatenate(
        [bqf.reshape(FC, 128), bkf.reshape(FC, 128)], axis=0
    ).T.copy()                                                # [128, 12]
    bvbc = np.broadcast_to(bv.astype(np.float32).reshape(D), (128, D)).copy()
    bobc = np.broadcast_to(bo.astype(np.float32).reshape(D), (128, D)).copy()

    return {
        "wqk": np.ascontiguousarray(wqk).astype(nbf),
        "wv": np.ascontiguousarray(wv3).astype(nbf),
        "wo": np.ascontiguousarray(wo3).astype(nbf),
        "bqk": np.ascontiguousarray(bqk),
        "bvbc": bvbc,
        "bobc": bobc,
    }


_NC_CACHE = {}


def kernel(x, Wq, bq, Wk, bk, Wv, bv, Wo, bo):
    x = np.asarray(x, dtype=np.float32)
    shared = _prep_shared(
        np.asarray(Wq), np.asarray(bq), np.asarray(Wk), np.asarray(bk),
        np.asarray(Wv), np.asarray(bv), np.asarray(Wo), np.asarray(bo))

    in_maps = []
    for c in range(NCORES):
        xc = x[c * BC : (c + 1) * BC]                    # [BC, S, D]
        xt = xc.transpose(2, 0, 1)                       # [D, BC, S]
        xt = xt.reshape(DC, 128, BC, S).astype(nbf)
        m = dict(shared)
        m["xt"] = np.ascontiguousarray(xt)
        in_maps.append(m)

    if "nc" not in _NC_CACHE:
        _NC_CACHE["nc"] = build_bass()
    nc = _NC_CACHE["nc"]

    res = run_bass_kernel_spmd(nc, in_maps, core_ids=list(range(NCORES)))
    out = np.concatenate([res.results[c]["out"] for c in range(NCORES)], axis=0)
    return out.astype(np.float32)


if __name__ == "__main__":
    rng = np.random.default_rng(0)
    ins = {
        "x": rng.standard_normal((B, S, D), dtype=np.float32),
        "Wq": rng.standard_normal((H, D, DH), dtype=np.float32) * 0.02,
        "bq": np.zeros((H, DH), np.float32),
        "Wk": rng.standard_normal((H, D, DH), dtype=np.float32) * 0.02,
        "bk": np.zeros((H, DH), np.float32),
        "Wv": rng.standard_normal((H, D, DH), dtype=np.float32) * 0.02,
        "bv": np.zeros((H, DH), np.float32),
        "Wo": rng.standard_normal((D, D), dtype=np.float32) * 0.02,
        "bo": np.zeros((D,), np.float32),
    }
    o = kernel(**ins)
    print("out", o.shape, o.dtype, float(np.abs(o).max()))
